# revision 1
# baseline (speedup 1.0000x reference)
"""Flat paged-attention (vLLM flat_pa, GQA, const-normalized softmax) on 8 TRN2 cores.

Sharding: data-parallel over decode sequences. Core c owns sequences
[8c, 8c+8) = 256 fetched blocks. The host gathers each core's K/V blocks
from the caches (the block_list indirection) and lays them out so the
device kernel is a dense stream:

  kt[h, d, (s,n,p)]  -- K gathered + transposed so head-dim is the SBUF
                        partition axis (QK^T contracts over d)
  vt[h, p, (s,n,d)]  -- V gathered, pos on partitions (PV contracts over pos)
  qt[d, (h,s,q)]     -- queries, scale baked in
  biast[p, (s,n,q)]  -- block bias with -CONST_VAL baked in, repeated over q

Per (head, seq): 32 K-stationary matmuls give scores^T [pos, 4q] in PSUM,
DVE adds bias, ACT exps, then 32 accumulating PV matmuls + 32 ones-column
matmuls give output [4, 128] and the group softmax denominator.
Division by the per-sequence denominator happens once at the end (valid
because the const-normalized softmax denominator is shared by all blocks
of a sequence).
"""

import sys

sys.path.insert(0, "/opt/trn_rl_repo")

import numpy as np

B = 64
BPS = 32           # blocks per sequence
BS = 128           # block size (tokens)
KVH = 8
QPK = 4            # q heads per kv head
HD = 128
NCORES = 8
SPC = 8            # sequences per core
BPC = SPC * BPS    # 256 blocks per core
CONST_VAL = 10.0
SCALE = 1.0 / np.sqrt(HD)
NEG = -30000.0

_NC_CACHE = {}


def build_nc(reps=1):
    """Build + compile the per-core Bass program. reps>1 wraps the body in a
    dynamic For_i loop (used only for timing)."""
    key = reps
    if key in _NC_CACHE:
        return _NC_CACHE[key]
    from concourse import bacc, mybir
    import concourse.tile as tile

    f32 = mybir.dt.float32
    nc = bacc.Bacc("TRN2", target_bir_lowering=False, debug=False, num_devices=NCORES)

    kt = nc.dram_tensor("kt", [KVH, HD, BPC * BS], f32, kind="ExternalInput")
    vt = nc.dram_tensor("vt", [KVH, BS, BPC * (HD + 1)], f32, kind="ExternalInput")
    qt = nc.dram_tensor("qt", [HD, KVH * SPC * QPK], f32, kind="ExternalInput")
    biast = nc.dram_tensor("biast", [BS, SPC * BPS * QPK], f32, kind="ExternalInput")
    out = nc.dram_tensor("out", [QPK, KVH * SPC * HD], f32, kind="ExternalOutput")

    CH = 2 * BPS * BS   # 8192 K cols per chunk (2 sequences)
    CHV = 2 * BPS * (HD + 1)  # V chunk incl. ones column per block

    with tile.TileContext(nc) as tc:
        from contextlib import ExitStack

        with ExitStack() as ctx:
            cpool = ctx.enter_context(tc.tile_pool(name="const", bufs=1))
            kpool = ctx.enter_context(tc.tile_pool(name="k", bufs=2))
            vpool = ctx.enter_context(tc.tile_pool(name="v", bufs=2))
            ppool = ctx.enter_context(tc.tile_pool(name="p", bufs=3))
            rpool = ctx.enter_context(tc.tile_pool(name="r", bufs=2))
            opool = ctx.enter_context(tc.tile_pool(name="osb", bufs=1))
            qkps = ctx.enter_context(tc.tile_pool(name="qkps", bufs=2, space="PSUM"))
            ops = ctx.enter_context(tc.tile_pool(name="ops", bufs=2, space="PSUM"))

            qt_sb = cpool.tile([HD, KVH * SPC * QPK], f32)
            nc.sync.dma_start(out=qt_sb[:], in_=qt[:])
            bias_sb = cpool.tile([BS, SPC * BPS * QPK], f32)
            nc.sync.dma_start(out=bias_sb[:], in_=biast[:])
            out_sb = opool.tile([QPK, KVH * SPC * HD], f32)

            def body():
                for h in range(KVH):
                    for sp in range(SPC // 2):
                        kch = kpool.tile([HD, CH], f32)
                        nc.sync.dma_start(
                            out=kch[:], in_=kt[h, :, sp * CH:(sp + 1) * CH]
                        )
                        vch = vpool.tile([BS, CHV], f32)
                        nc.sync.dma_start(
                            out=vch[:], in_=vt[h, :, sp * CHV:(sp + 1) * CHV]
                        )
                        for sl in range(2):
                            s = sp * 2 + sl
                            qk = qkps.tile([BS, BPS * QPK], f32)
                            qcol = (h * SPC + s) * QPK
                            for nl in range(BPS):
                                nc.tensor.matmul(
                                    out=qk[:, nl * QPK:(nl + 1) * QPK],
                                    lhsT=kch[:, (sl * BPS + nl) * BS:(sl * BPS + nl + 1) * BS],
                                    rhs=qt_sb[:, qcol:qcol + QPK],
                                    start=True,
                                    stop=True,
                                )
                            pb = ppool.tile([BS, BPS * QPK], f32, tag="pb")
                            nc.vector.tensor_add(
                                out=pb[:],
                                in0=qk[:],
                                in1=bias_sb[:, s * BPS * QPK:(s + 1) * BPS * QPK],
                            )
                            pe = ppool.tile([BS, BPS * QPK], f32, tag="pe")
                            nc.scalar.activation(
                                pe[:], pb[:], mybir.ActivationFunctionType.Exp
                            )
                            o_ps = ops.tile([QPK, HD + 1], f32)
                            for nl in range(BPS):
                                b = sl * BPS + nl
                                nc.tensor.matmul(
                                    out=o_ps[:],
                                    lhsT=pe[:, nl * QPK:(nl + 1) * QPK],
                                    rhs=vch[:, b * (HD + 1):(b + 1) * (HD + 1)],
                                    start=(nl == 0),
                                    stop=(nl == BPS - 1),
                                )
                            rec = rpool.tile([QPK, 1], f32)
                            nc.vector.reciprocal(rec[:], o_ps[:, HD:HD + 1])
                            nc.vector.tensor_scalar_mul(
                                out_sb[:, (h * SPC + s) * HD:(h * SPC + s + 1) * HD],
                                o_ps[:, 0:HD],
                                rec[:],
                            )
                nc.sync.dma_start(out=out[:], in_=out_sb[:])

            if reps == 1:
                body()
            else:
                with tc.For_i(0, reps, 1):
                    body()

    nc.compile()
    _NC_CACHE[key] = nc
    return nc


def prep_inputs(query, key_cache, value_cache, block_list, block_mapping,
                block_bias, block_groups):
    """Host-side shard + gather + layout. Returns per-core in_maps."""
    query = np.asarray(query, dtype=np.float32)
    key_cache = np.asarray(key_cache, dtype=np.float32)
    value_cache = np.asarray(value_cache, dtype=np.float32)
    block_list = np.asarray(block_list)
    block_bias = np.asarray(block_bias, dtype=np.float32)
    block_groups = np.asarray(block_groups)

    # per-sequence fetched-block rows (pad to BPS with masked dummies)
    seq_rows = np.zeros((B, BPS), dtype=np.int64)
    pad_mask = np.zeros((B, BPS), dtype=bool)
    for s in range(B):
        rows = np.flatnonzero(block_groups == s)
        assert len(rows) <= BPS, f"sequence {s} has {len(rows)} > {BPS} blocks"
        seq_rows[s, :len(rows)] = rows
        pad_mask[s, len(rows):] = True

    qs = (query.reshape(B, KVH, QPK, HD) * SCALE)  # (s, h, q, d)

    in_maps = []
    for c in range(NCORES):
        rows = seq_rows[c * SPC:(c + 1) * SPC].reshape(-1)          # [256]
        pmask = pad_mask[c * SPC:(c + 1) * SPC].reshape(-1)         # [256]
        bl = block_list[rows].astype(np.int64)
        gk = key_cache[bl]                                           # [256,p,h,d]
        gv = value_cache[bl]
        kt_c = np.ascontiguousarray(gk.transpose(2, 3, 0, 1)).reshape(KVH, HD, -1)
        gv = np.concatenate(
            [gv, np.ones((BPC, BS, KVH, 1), dtype=np.float32)], axis=3)
        vt_c = np.ascontiguousarray(gv.transpose(2, 1, 0, 3)).reshape(KVH, BS, -1)
        # queries for this core: (d, h, s, q)
        qt_c = np.ascontiguousarray(
            qs[c * SPC:(c + 1) * SPC].transpose(3, 1, 0, 2)
        ).reshape(HD, -1)
        # bias - CONST_VAL, padded blocks fully masked, repeated over q: (p,(s,n,q))
        bia = block_bias[rows] - CONST_VAL                           # [256, p]
        bia[pmask] = NEG - CONST_VAL
        biast_c = np.ascontiguousarray(
            np.repeat(bia.T[:, :, None], QPK, axis=2)
        ).reshape(BS, -1)
        in_maps.append({
            "kt": kt_c, "vt": vt_c,
            "qt": np.ascontiguousarray(qt_c, dtype=np.float32),
            "biast": biast_c.astype(np.float32),
        })
    return in_maps


def assemble_output(results):
    out = np.zeros((B, KVH * QPK, HD), dtype=np.float32)
    for c in range(NCORES):
        o = results[c]["out"].reshape(QPK, KVH, SPC, HD)  # (q,h,s,d)
        out[c * SPC:(c + 1) * SPC] = o.transpose(2, 1, 0, 3).reshape(SPC, KVH * QPK, HD)
    return out


def kernel(query, key_cache, value_cache, block_list, block_mapping,
           block_bias, block_groups):
    from concourse.bass_utils import run_bass_kernel_spmd

    nc = build_nc(reps=1)
    in_maps = prep_inputs(query, key_cache, value_cache, block_list,
                          block_mapping, block_bias, block_groups)
    res = run_bass_kernel_spmd(nc, in_maps, core_ids=list(range(NCORES)))
    return assemble_output(res.results)



# revision 4
# speedup vs baseline: 2.3825x; 2.3825x over previous
"""Flat paged-attention (vLLM flat_pa, GQA, const-normalized softmax) on 8 TRN2 cores.

Sharding: data-parallel over decode sequences. Core c owns sequences
[8c, 8c+8) = 256 fetched blocks. The host gathers each core's K/V blocks
from the caches (the block_list indirection), converts to bf16, and lays
them out so the device kernel is a dense stream:

  kt[h, d, (s,n,p)]  -- K gathered + transposed so head-dim is the SBUF
                        partition axis (QK^T contracts over d), bf16
  vt[h, p, (s,n,d+1)] -- V gathered, pos on partitions (PV contracts over
                        pos), bf16. Column d==HD holds exp(block_bias)
                        (the softmax-denominator "ones" column), and all
                        HD value columns are pre-scaled by exp(block_bias)
                        so masked slots contribute exactly zero to both
                        numerator and denominator -- no bias work on device.
  qt[d, (h,s,q)]     -- queries, scale baked in, bf16

Per (head, seq): 32 K-stationary matmuls give scores^T [pos, 32*4q] in
PSUM, one ACT op computes exp(x - CONST_VAL) into bf16 SBUF, then 32
accumulating PV matmuls give output [4, HD+1] where the last column is
the group softmax denominator. Division by the per-sequence denominator
happens once per (head, seq) (valid because the const-normalized softmax
denominator is shared by all blocks of a sequence).
"""

import sys

sys.path.insert(0, "/opt/trn_rl_repo")

import numpy as np
import ml_dtypes

BF16 = ml_dtypes.bfloat16

B = 64
BPS = 32           # blocks per sequence
BS = 128           # block size (tokens)
KVH = 8
QPK = 4            # q heads per kv head
HD = 128
NCORES = 8
SPC = 8            # sequences per core
BPC = SPC * BPS    # 256 blocks per core
CONST_VAL = 10.0
SCALE = 1.0 / np.sqrt(HD)

_NC_CACHE = {}


def build_nc(reps=1):
    """Build + compile the per-core Bass program. reps>1 wraps the body in a
    dynamic For_i loop (used only for timing)."""
    key = reps
    if key in _NC_CACHE:
        return _NC_CACHE[key]
    from concourse import bacc, mybir
    import concourse.tile as tile

    f32 = mybir.dt.float32
    bf16 = mybir.dt.bfloat16
    nc = bacc.Bacc("TRN2", target_bir_lowering=False, debug=False, num_devices=NCORES)

    kt = nc.dram_tensor("kt", [KVH, HD, BPC * BS], bf16, kind="ExternalInput")
    vt = nc.dram_tensor("vt", [KVH, BS, BPC * (HD + 1)], bf16, kind="ExternalInput")
    qt = nc.dram_tensor("qt", [HD, KVH * SPC * QPK], bf16, kind="ExternalInput")
    out = nc.dram_tensor("out", [QPK, KVH * SPC * HD], f32, kind="ExternalOutput")

    CH = 2 * BPS * BS   # 8192 K cols per chunk (2 sequences)
    CHV = 2 * BPS * (HD + 1)  # V chunk incl. denominator column per block

    with tile.TileContext(nc) as tc:
        from contextlib import ExitStack

        with ExitStack() as ctx:
            cpool = ctx.enter_context(tc.tile_pool(name="const", bufs=1))
            kpool = ctx.enter_context(tc.tile_pool(name="k", bufs=3))
            vpool = ctx.enter_context(tc.tile_pool(name="v", bufs=3))
            ppool = ctx.enter_context(tc.tile_pool(name="p", bufs=3))
            rpool = ctx.enter_context(tc.tile_pool(name="r", bufs=2))
            opool = ctx.enter_context(tc.tile_pool(name="osb", bufs=1))
            qkps = ctx.enter_context(tc.tile_pool(name="qkps", bufs=2, space="PSUM"))
            ops = ctx.enter_context(tc.tile_pool(name="ops", bufs=2, space="PSUM"))

            qt_sb = cpool.tile([HD, KVH * SPC * QPK], bf16)
            nc.sync.dma_start(out=qt_sb[:], in_=qt[:])
            negc = cpool.tile([BS, 1], f32)
            nc.gpsimd.memset(negc[:], -CONST_VAL)
            out_sb = opool.tile([QPK, KVH * SPC * HD], f32)

            def body():
                for h in range(KVH):
                    for sp in range(SPC // 2):
                        kch = kpool.tile([HD, CH], bf16)
                        nc.sync.dma_start(
                            out=kch[:], in_=kt[h, :, sp * CH:(sp + 1) * CH]
                        )
                        vch = vpool.tile([BS, CHV], bf16)
                        nc.sync.dma_start(
                            out=vch[:], in_=vt[h, :, sp * CHV:(sp + 1) * CHV]
                        )
                        for sl in range(2):
                            s = sp * 2 + sl
                            qk = qkps.tile([BS, BPS * QPK], f32)
                            qcol = (h * SPC + s) * QPK
                            for nl in range(BPS):
                                nc.tensor.matmul(
                                    out=qk[:, nl * QPK:(nl + 1) * QPK],
                                    lhsT=kch[:, (sl * BPS + nl) * BS:(sl * BPS + nl + 1) * BS],
                                    rhs=qt_sb[:, qcol:qcol + QPK],
                                    start=True,
                                    stop=True,
                                )
                            pe = ppool.tile([BS, BPS * QPK], bf16, tag="pe")
                            nc.scalar.activation(
                                pe[:], qk[:], mybir.ActivationFunctionType.Exp,
                                bias=negc[:],
                            )
                            o_ps = ops.tile([QPK, HD + 1], f32)
                            for nl in range(BPS):
                                b = sl * BPS + nl
                                nc.tensor.matmul(
                                    out=o_ps[:],
                                    lhsT=pe[:, nl * QPK:(nl + 1) * QPK],
                                    rhs=vch[:, b * (HD + 1):(b + 1) * (HD + 1)],
                                    start=(nl == 0),
                                    stop=(nl == BPS - 1),
                                )
                            rec = rpool.tile([QPK, 1], f32)
                            nc.vector.reciprocal(rec[:], o_ps[:, HD:HD + 1])
                            nc.vector.tensor_scalar_mul(
                                out_sb[:, (h * SPC + s) * HD:(h * SPC + s + 1) * HD],
                                o_ps[:, 0:HD],
                                rec[:],
                            )
                nc.sync.dma_start(out=out[:], in_=out_sb[:])

            if reps == 1:
                body()
            else:
                with tc.For_i(0, reps, 1):
                    body()

    nc.compile()
    _NC_CACHE[key] = nc
    return nc


def prep_inputs(query, key_cache, value_cache, block_list, block_mapping,
                block_bias, block_groups):
    """Host-side shard + gather + layout + bf16 conversion. Returns per-core
    in_maps."""
    query = np.asarray(query, dtype=np.float32)
    key_cache = np.asarray(key_cache, dtype=np.float32)
    value_cache = np.asarray(value_cache, dtype=np.float32)
    block_list = np.asarray(block_list)
    block_bias = np.asarray(block_bias, dtype=np.float32)
    block_groups = np.asarray(block_groups)

    # per-sequence fetched-block rows (pad to BPS with masked dummies)
    seq_rows = np.zeros((B, BPS), dtype=np.int64)
    pad_mask = np.zeros((B, BPS), dtype=bool)
    for s in range(B):
        rows = np.flatnonzero(block_groups == s)
        assert len(rows) <= BPS, f"sequence {s} has {len(rows)} > {BPS} blocks"
        seq_rows[s, :len(rows)] = rows
        pad_mask[s, len(rows):] = True

    qs = (query.reshape(B, KVH, QPK, HD) * SCALE)  # (s, h, q, d)

    in_maps = []
    for c in range(NCORES):
        rows = seq_rows[c * SPC:(c + 1) * SPC].reshape(-1)          # [256]
        pmask = pad_mask[c * SPC:(c + 1) * SPC].reshape(-1)         # [256]
        bl = block_list[rows].astype(np.int64)
        gk = key_cache[bl]                                           # [256,p,h,d]
        kt_c = np.ascontiguousarray(
            gk.transpose(2, 3, 0, 1).astype(BF16)).reshape(KVH, HD, -1)
        # exp(bias) mask: 1 for live slots, 0 for masked/padded slots
        # (exact for bias in {0, -30000}); scales V and forms the
        # denominator column, so masked slots contribute exactly 0.
        m = np.exp(block_bias[rows])                                 # [256, p]
        m[pmask] = 0.0
        gv = value_cache[bl] * m[:, :, None, None]                   # [256,p,h,d]
        gv = np.concatenate(
            [gv, np.broadcast_to(m[:, :, None, None], (BPC, BS, KVH, 1))],
            axis=3)
        vt_c = np.ascontiguousarray(
            gv.transpose(2, 1, 0, 3).astype(BF16)).reshape(KVH, BS, -1)
        # queries for this core: (d, h, s, q)
        qt_c = np.ascontiguousarray(
            qs[c * SPC:(c + 1) * SPC].transpose(3, 1, 0, 2).astype(BF16)
        ).reshape(HD, -1)
        in_maps.append({"kt": kt_c, "vt": vt_c, "qt": qt_c})
    return in_maps


def assemble_output(results):
    out = np.zeros((B, KVH * QPK, HD), dtype=np.float32)
    for c in range(NCORES):
        o = results[c]["out"].reshape(QPK, KVH, SPC, HD)  # (q,h,s,d)
        out[c * SPC:(c + 1) * SPC] = o.transpose(2, 1, 0, 3).reshape(SPC, KVH * QPK, HD)
    return out


def kernel(query, key_cache, value_cache, block_list, block_mapping,
           block_bias, block_groups):
    from concourse.bass_utils import run_bass_kernel_spmd

    nc = build_nc(reps=1)
    in_maps = prep_inputs(query, key_cache, value_cache, block_list,
                          block_mapping, block_bias, block_groups)
    res = run_bass_kernel_spmd(nc, in_maps, core_ids=list(range(NCORES)))
    return assemble_output(res.results)


# revision 8
# speedup vs baseline: 17.8747x; 7.5026x over previous
"""Flat paged-attention (vLLM flat_pa, GQA, const-normalized softmax) on 8 TRN2 cores.

Sharding: data-parallel over decode sequences. Core c owns sequences
[8c, 8c+8) = 256 fetched blocks. The host gathers each core's K/V blocks
from the caches (the block_list indirection), converts to bf16, and lays
them out so the device kernel is a dense stream:

  kt[h, d, (s,n,p)]  -- K gathered + transposed so head-dim is the SBUF
                        partition axis (QK^T contracts over d), bf16
  vt[h, p, (s,n,d+1)] -- V gathered, pos on partitions (PV contracts over
                        pos), bf16. Column d==HD holds exp(block_bias)
                        (the softmax-denominator "ones" column), and all
                        HD value columns are pre-scaled by exp(block_bias)
                        so masked slots contribute exactly zero to both
                        numerator and denominator -- no bias work on device.
  qt[d, (h,s,q)]     -- queries, scale baked in, bf16

Per (head, seq): 32 K-stationary matmuls give scores^T [pos, 32*4q] in
PSUM, one ACT op computes exp(x - CONST_VAL) into bf16 SBUF, then 32
accumulating PV matmuls give output [4, HD+1] where the last column is
the group softmax denominator. Division by the per-sequence denominator
happens once per (head, seq) (valid because the const-normalized softmax
denominator is shared by all blocks of a sequence).
"""

import sys

sys.path.insert(0, "/opt/trn_rl_repo")

import numpy as np
import ml_dtypes

BF16 = ml_dtypes.bfloat16

B = 64
BPS = 32           # blocks per sequence
BS = 128           # block size (tokens)
KVH = 8
QPK = 4            # q heads per kv head
HD = 128
NCORES = 8
SPC = 8            # sequences per core
BPC = SPC * BPS    # 256 blocks per core
CONST_VAL = 10.0
SCALE = 1.0 / np.sqrt(HD)

_NC_CACHE = {}


def build_nc(reps=1, variant="full"):
    """Build + compile the per-core Bass program. reps>1 wraps the body in a
    dynamic For_i loop (used only for timing). variant: "full" (the real
    kernel), "dmaonly" (just the HBM streams), "computeonly" (one chunk
    DMA, full compute against it) — the latter two only for perf triage."""
    key = (reps, variant)
    if key in _NC_CACHE:
        return _NC_CACHE[key]
    from concourse import bacc, mybir
    import concourse.tile as tile

    f32 = mybir.dt.float32
    bf16 = mybir.dt.bfloat16
    nc = bacc.Bacc("TRN2", target_bir_lowering=False, debug=False, num_devices=NCORES)

    kt = nc.dram_tensor("kt", [KVH, HD, BPC * BS], bf16, kind="ExternalInput")
    vt = nc.dram_tensor("vt", [KVH, BS, BPC * (HD + 1)], bf16, kind="ExternalInput")
    qt = nc.dram_tensor("qt", [HD, KVH * SPC * QPK], bf16, kind="ExternalInput")
    out = nc.dram_tensor("out", [QPK, KVH * SPC * HD], f32, kind="ExternalOutput")

    CH = 2 * BPS * BS   # 8192 K cols per chunk (2 sequences)
    CHV = 2 * BPS * (HD + 1)  # V chunk incl. denominator column per block

    with tile.TileContext(nc) as tc:
        from contextlib import ExitStack

        with ExitStack() as ctx:
            cpool = ctx.enter_context(tc.tile_pool(name="const", bufs=1))
            kpool = ctx.enter_context(tc.tile_pool(name="k", bufs=3))
            vpool = ctx.enter_context(tc.tile_pool(name="v", bufs=3))
            ppool = ctx.enter_context(tc.tile_pool(name="p", bufs=3))
            rpool = ctx.enter_context(tc.tile_pool(name="r", bufs=2))
            opool = ctx.enter_context(tc.tile_pool(name="osb", bufs=1))
            qkps = ctx.enter_context(tc.tile_pool(name="qkps", bufs=2, space="PSUM"))
            ops = ctx.enter_context(tc.tile_pool(name="ops", bufs=2, space="PSUM"))

            qt_sb = cpool.tile([HD, KVH * SPC * QPK], bf16)
            nc.sync.dma_start(out=qt_sb[:], in_=qt[:])
            negc = cpool.tile([BS, 1], f32)
            nc.gpsimd.memset(negc[:], -CONST_VAL)
            out_sb = opool.tile([QPK, KVH * SPC * HD], f32)
            if variant == "dmaonly":
                nc.gpsimd.memset(out_sb[:], 0.0)

            if variant == "computeonly":
                kch0 = cpool.tile([HD, CH], bf16)
                nc.sync.dma_start(out=kch0[:], in_=kt[0, :, 0:CH])
                vch0 = cpool.tile([BS, CHV], bf16)
                nc.sync.dma_start(out=vch0[:], in_=vt[0, :, 0:CHV])

            def body():
                # software pipeline over (h, sp, sl) units: per unit emit
                # QK(i) -> ACT(i) -> PV(i-1), so PV never stalls the PE
                # waiting on the same unit's exp (ACT(i-1) ran under QK(i)).
                units = [(h, sp, sl)
                         for h in range(KVH)
                         for sp in range(SPC // 2)
                         for sl in range(2)]
                chunks = {}   # (h, sp) -> (kch, vch)
                pend = [None, None]  # pe tile + metadata for units i-1

                def emit_pv(p):
                    pe_t, vch_t, sl_t, ocol = p
                    o_ps = ops.tile([QPK, HD + 1], f32)
                    for nl in range(BPS):
                        b = sl_t * BPS + nl
                        nc.tensor.matmul(
                            out=o_ps[:],
                            lhsT=pe_t[:, nl * QPK:(nl + 1) * QPK],
                            rhs=vch_t[:, b * (HD + 1):(b + 1) * (HD + 1)],
                            start=(nl == 0),
                            stop=(nl == BPS - 1),
                        )
                    rec = rpool.tile([QPK, 1], f32)
                    nc.vector.reciprocal(rec[:], o_ps[:, HD:HD + 1])
                    nc.vector.tensor_scalar_mul(
                        out_sb[:, ocol * HD:(ocol + 1) * HD],
                        o_ps[:, 0:HD],
                        rec[:],
                    )

                for i, (h, sp, sl) in enumerate(units):
                    if sl == 0:
                        if variant == "computeonly":
                            chunks[(h, sp)] = (kch0, vch0)
                        else:
                            kch = kpool.tile([HD, CH], bf16)
                            nc.sync.dma_start(
                                out=kch[:], in_=kt[h, :, sp * CH:(sp + 1) * CH]
                            )
                            vch = vpool.tile([BS, CHV], bf16)
                            nc.sync.dma_start(
                                out=vch[:], in_=vt[h, :, sp * CHV:(sp + 1) * CHV]
                            )
                            chunks[(h, sp)] = (kch, vch)
                    if variant == "dmaonly":
                        continue
                    kch, vch = chunks[(h, sp)]
                    s = sp * 2 + sl
                    qk = qkps.tile([BS, BPS * QPK], f32)
                    qcol = (h * SPC + s) * QPK
                    for nl in range(BPS):
                        nc.tensor.matmul(
                            out=qk[:, nl * QPK:(nl + 1) * QPK],
                            lhsT=kch[:, (sl * BPS + nl) * BS:(sl * BPS + nl + 1) * BS],
                            rhs=qt_sb[:, qcol:qcol + QPK],
                            start=True,
                            stop=True,
                        )
                    pe = ppool.tile([BS, BPS * QPK], bf16, tag="pe")
                    nc.scalar.activation(
                        pe[:], qk[:], mybir.ActivationFunctionType.Exp,
                        bias=negc[:],
                    )
                    if pend[0] is not None:
                        emit_pv(pend[0])
                    pend[0] = (pe, vch, sl, h * SPC + s)
                if pend[0] is not None:
                    emit_pv(pend[0])
                nc.sync.dma_start(out=out[:], in_=out_sb[:])

            if reps == 1:
                body()
            else:
                with tc.For_i(0, reps, 1):
                    body()

    nc.compile()
    _NC_CACHE[key] = nc
    return nc


def prep_inputs(query, key_cache, value_cache, block_list, block_mapping,
                block_bias, block_groups):
    """Host-side shard + gather + layout + bf16 conversion. Returns per-core
    in_maps."""
    query = np.asarray(query, dtype=np.float32)
    key_cache = np.asarray(key_cache, dtype=np.float32)
    value_cache = np.asarray(value_cache, dtype=np.float32)
    block_list = np.asarray(block_list)
    block_bias = np.asarray(block_bias, dtype=np.float32)
    block_groups = np.asarray(block_groups)

    # per-sequence fetched-block rows (pad to BPS with masked dummies)
    seq_rows = np.zeros((B, BPS), dtype=np.int64)
    pad_mask = np.zeros((B, BPS), dtype=bool)
    for s in range(B):
        rows = np.flatnonzero(block_groups == s)
        assert len(rows) <= BPS, f"sequence {s} has {len(rows)} > {BPS} blocks"
        seq_rows[s, :len(rows)] = rows
        pad_mask[s, len(rows):] = True

    qs = (query.reshape(B, KVH, QPK, HD) * SCALE)  # (s, h, q, d)

    in_maps = []
    for c in range(NCORES):
        rows = seq_rows[c * SPC:(c + 1) * SPC].reshape(-1)          # [256]
        pmask = pad_mask[c * SPC:(c + 1) * SPC].reshape(-1)         # [256]
        bl = block_list[rows].astype(np.int64)
        gk = key_cache[bl]                                           # [256,p,h,d]
        kt_c = np.ascontiguousarray(
            gk.transpose(2, 3, 0, 1).astype(BF16)).reshape(KVH, HD, -1)
        # exp(bias) mask: 1 for live slots, 0 for masked/padded slots
        # (exact for bias in {0, -30000}); scales V and forms the
        # denominator column, so masked slots contribute exactly 0.
        m = np.exp(block_bias[rows])                                 # [256, p]
        m[pmask] = 0.0
        gv = value_cache[bl] * m[:, :, None, None]                   # [256,p,h,d]
        gv = np.concatenate(
            [gv, np.broadcast_to(m[:, :, None, None], (BPC, BS, KVH, 1))],
            axis=3)
        vt_c = np.ascontiguousarray(
            gv.transpose(2, 1, 0, 3).astype(BF16)).reshape(KVH, BS, -1)
        # queries for this core: (d, h, s, q)
        qt_c = np.ascontiguousarray(
            qs[c * SPC:(c + 1) * SPC].transpose(3, 1, 0, 2).astype(BF16)
        ).reshape(HD, -1)
        in_maps.append({"kt": kt_c, "vt": vt_c, "qt": qt_c})
    return in_maps


def assemble_output(results):
    out = np.zeros((B, KVH * QPK, HD), dtype=np.float32)
    for c in range(NCORES):
        o = results[c]["out"].reshape(QPK, KVH, SPC, HD)  # (q,h,s,d)
        out[c * SPC:(c + 1) * SPC] = o.transpose(2, 1, 0, 3).reshape(SPC, KVH * QPK, HD)
    return out


def kernel(query, key_cache, value_cache, block_list, block_mapping,
           block_bias, block_groups):
    from concourse.bass_utils import run_bass_kernel_spmd

    nc = build_nc(reps=1)
    in_maps = prep_inputs(query, key_cache, value_cache, block_list,
                          block_mapping, block_bias, block_groups)
    res = run_bass_kernel_spmd(nc, in_maps, core_ids=list(range(NCORES)))
    return assemble_output(res.results)
